# revision 16
# baseline (speedup 1.0000x reference)
"""Multi-head attention (B=4,S=2048,H=1024,NH=16,D=64) on 8 trn2 cores.

Sharding: core c = (g, b) with g = c // 4 (head-group of 8 heads = 512 dims,
tensor parallel) and b = c % 4 (batch, data parallel). Each core computes a
partial output (its head-group's contribution to the final projection),
transposed: ot = (attn_out_g @ wo_g)^T of shape [H, S]. Host sums the two
group partials per batch and adds bias.

Math notes (host/device split):
  - k-proj bias bk drops out of softmax (adds a per-query constant along the
    key axis), so it is not applied on device.
  - v-proj bias bv commutes through normalized attention (rows of the score
    matrix sum to 1): its contribution is bv @ wo, folded into the output
    bias on the host.

On-device layout: everything is computed transposed (feature dim on
partitions, sequence on the free axis) so the softmax key-axis lands on
partitions. Scores S^T are built per head as K_h^T(stationary) x Q_h^T,
exp() runs on the scalar engine straight out of PSUM, and the ones-column
appended to V in the AV matmul yields the softmax denominators for free.

Schedule: the kernel is ACT-bound (256 exp instructions ~ 293us floor);
emission is one global pipeline: projections are split into per-qc units
interleaved with the first attention call's kt-steps (ramp), all attention
(t, qcp) calls share one global pending-AV queue so call boundaries stitch
without ACT gaps, and softmax normalization drains PSUM immediately (DVE
copy) then computes the reciprocal wide ([128,8] via a DRAM transpose
round-trip) off the critical path.
"""

import sys

if "/opt/trn_rl_repo" not in sys.path:
    sys.path.insert(0, "/opt/trn_rl_repo")

from collections import deque

import numpy as np

B, S, H, NH, D = 4, 2048, 1024, 16, 64
G = 2  # head-group split across cores (tensor parallel axis)
GH = H // G  # 512 dims (8 heads) per group
NCORES = 8
SCALE = 1.0 / float(D) ** 0.5  # 1/8

KT = H // 128  # 8 contraction tiles for projections
MT = GH // 128  # 4 m-tiles = head pairs per group
NQC = S // 512  # 4 sequence chunks of 512
SQ = S // 128  # 16 key-sequence tiles
VW = D + 1  # 65: V columns + ones column per head

_CACHE = {}

CFG = {
    "xs_bufs": 32,
    "pt_bufs": 12,
    "av_depth": 4,  # pending-AV queue depth (global, in kt-steps)
    "prefetch_ahead": 6,  # strip-DMA prefetch distance, in kt-steps
    "ou_bufs": 3,
    "bc_bufs": 2,
}


def _build():
    import concourse.tile as tile
    from concourse import bacc, mybir

    F32 = mybir.dt.float32
    F16 = mybir.dt.float16
    AF = mybir.ActivationFunctionType
    OP = mybir.AluOpType

    nc = bacc.Bacc("TRN2", target_bir_lowering=False, debug=False)

    xq = nc.dram_tensor("xq", [H, S], F16, kind="ExternalInput")
    xk = nc.dram_tensor("xk", [H, S], F16, kind="ExternalInput")
    xv = nc.dram_tensor("xv", [H, S], F16, kind="ExternalInput")
    wqd = nc.dram_tensor("wq", [H, GH], F16, kind="ExternalInput")
    wkd = nc.dram_tensor("wk", [H, GH], F16, kind="ExternalInput")
    wvd = nc.dram_tensor("wv", [H, GH], F16, kind="ExternalInput")
    wod = nc.dram_tensor("wo", [GH, H], F32, kind="ExternalInput")
    bqd = nc.dram_tensor("bq", [GH], F32, kind="ExternalInput")
    otd = nc.dram_tensor("ot", [H, S], F32, kind="ExternalOutput")

    with tile.TileContext(nc) as tc:
        with (
            tc.tile_pool(name="res", bufs=1) as res,
            tc.tile_pool(name="rot", bufs=2) as rot,
            tc.tile_pool(name="psmm", bufs=2, space="PSUM") as psmm,
            tc.tile_pool(name="pso", bufs=2, space="PSUM") as pso,
            tc.tile_pool(name="dsc", bufs=8, space="DRAM") as dsc,
        ):
            # ---- residents (fine-grained for emission-time dep tracking) ----
            qhT = [
                [
                    res.tile([128, 512], F16, tag=f"qhT{m}_{qc}", name=f"qhT{m}_{qc}")
                    for qc in range(4)
                ]
                for m in range(MT)
            ]
            khT = [
                [
                    res.tile([128, 512], F16, tag=f"khT{m}_{qc}", name=f"khT{m}_{qc}")
                    for qc in range(4)
                ]
                for m in range(MT)
            ]
            oT = [
                [
                    res.tile([128, 1024], F16, tag=f"oT{t}_{qcp}", name=f"oT{t}_{qcp}")
                    for qcp in range(2)
                ]
                for t in range(MT)
            ]
            vaug = [
                res.tile([128, 8 * VW], F16, tag=f"vaug{kb}", name=f"vaug{kb}")
                for kb in range(SQ)
            ]
            wo_bf = [
                res.tile([128, H], F16, tag=f"wob{t}", name=f"wob{t}")
                for t in range(MT)
            ]
            wq_sb = [
                res.tile([128, GH], F16, tag=f"wq{kt}", name=f"wq{kt}")
                for kt in range(KT)
            ]
            wk_sb = [
                res.tile([128, GH], F16, tag=f"wk{kt}", name=f"wk{kt}")
                for kt in range(KT)
            ]
            wv_sb = [
                res.tile([128, GH], F16, tag=f"wv{kt}", name=f"wv{kt}")
                for kt in range(KT)
            ]
            bq_sb = res.tile([128, MT], F32, tag="bqsb", name="bq_sb")

            # ---- early staging, ordered by first use: the first bursts are
            # K(0,0)/Q(0,0)/Q(1,0), so wk/wq land first; wv follows (V bursts
            # start at step 4); wo staging is deferred into the weave.
            for kt in range(KT):
                nc.scalar.dma_start(
                    out=wk_sb[kt], in_=wkd.ap()[kt * 128 : (kt + 1) * 128, :]
                )
            for kt in range(KT):
                nc.scalar.dma_start(
                    out=wq_sb[kt], in_=wqd.ap()[kt * 128 : (kt + 1) * 128, :]
                )
            for m in range(MT):
                nc.scalar.dma_start(
                    out=bq_sb[:, m : m + 1],
                    in_=bqd.ap()[m * 128 : (m + 1) * 128].rearrange(
                        "(p o) -> p o", o=1
                    ),
                )
            for kt in range(KT):
                nc.scalar.dma_start(
                    out=wv_sb[kt], in_=wvd.ap()[kt * 128 : (kt + 1) * 128, :]
                )
            # ones columns of the augmented-V tiles (V slots overwritten later)
            for kb in range(SQ):
                nc.vector.memset(vaug[kb], 1.0)

            def stage_wo():
                for t in range(MT):
                    wos = rot.tile([128, H], F32, tag="wos", bufs=2, name=f"wos{t}")
                    nc.sync.dma_start(
                        out=wos, in_=wod.ap()[t * 128 : (t + 1) * 128, :]
                    )
                    nc.vector.tensor_copy(wo_bf[t], wos)

            def load_strips(xd, qc):
                xs = []
                for kt in range(KT):
                    st = rot.tile(
                        [128, 512], F16, tag="xs", bufs=CFG["xs_bufs"], name=f"xs{kt}"
                    )
                    nc.sync.dma_start(
                        out=st,
                        in_=xd.ap()[
                            kt * 128 : (kt + 1) * 128, qc * 512 : (qc + 1) * 512
                        ],
                    )
                    xs.append(st)
                return xs

            # ---------- projection unit emitters (one psum burst each) ----------
            def v_burst(xs, qc, sql):
                kb = qc * 4 + sql
                ps = psmm.tile([128, 1024], F32, tag="mm", name=f"psv{kb}")
                for kt in range(KT):
                    nc.tensor.matmul(
                        ps[:, 0:512],
                        lhsT=xs[kt][:, sql * 128 : (sql + 1) * 128],
                        rhs=wv_sb[kt],
                        start=(kt == 0),
                        stop=(kt == KT - 1),
                    )
                for h in range(8):
                    nc.vector.tensor_copy(
                        vaug[kb][:, h * VW : h * VW + D],
                        ps[:, h * D : (h + 1) * D],
                    )

            def k_burst(xs, qc, m):
                ps = psmm.tile([128, 1024], F32, tag="mm", name=f"psk{m}")
                for kt in range(KT):
                    nc.tensor.matmul(
                        ps[:, 0:512],
                        lhsT=wk_sb[kt][:, m * 128 : (m + 1) * 128],
                        rhs=xs[kt],
                        start=(kt == 0),
                        stop=(kt == KT - 1),
                    )
                nc.vector.tensor_copy(khT[m][qc], ps[:, 0:512])

            def q_burst(xs, qc, m):
                ps = psmm.tile([128, 1024], F32, tag="mm", name=f"psq{m}")
                for kt in range(KT):
                    nc.tensor.matmul(
                        ps[:, 0:512],
                        lhsT=wq_sb[kt][:, m * 128 : (m + 1) * 128],
                        rhs=xs[kt],
                        start=(kt == 0),
                        stop=(kt == KT - 1),
                    )
                nc.vector.tensor_scalar(
                    qhT[m][qc], ps[:, 0:512], bq_sb[:, m : m + 1], None, OP.add
                )

            def proj_units(qc):
                """The 12 projection bursts for one 512-seq chunk, as
                (prefetch, burst) thunk pairs. Each burst has its own strip
                load (bursts are woven far apart, so sharing strips across
                bursts would stretch xs-ring lifetimes into dependency
                cycles); the prefetch is emitted several steps ahead so the
                1MB strip DMA never sits on the QK critical path. V's four
                bursts are adjacent in the weave and share one load."""
                us = []
                vstate = {}

                def mk(kind, m):
                    slot = {}

                    def prefetch():
                        if kind == "v":
                            if "xs" not in vstate:
                                vstate["xs"] = load_strips(xv, qc)
                        else:
                            xd = xk if kind == "k" else xq
                            slot["xs"] = load_strips(xd, qc)

                    def run():
                        if kind == "v":
                            v_burst(vstate["xs"], qc, m)
                        elif kind == "k":
                            k_burst(slot["xs"], qc, m)
                        else:
                            q_burst(slot["xs"], qc, m)

                    return prefetch, run

                for kind in ("v", "k", "q"):
                    for m in range(4):
                        us.append(mk(kind, m))
                return us

            # ---------- out-projection (one 128-row output block) ----------
            def o_burst(qcp, qcc, m):
                ps = psmm.tile([128, 1024], F32, tag="mm", name=f"pso{m}")
                for t in range(MT):
                    nc.tensor.matmul(
                        ps[:, 0:512],
                        lhsT=wo_bf[t][:, m * 128 : (m + 1) * 128],
                        rhs=oT[t][qcp][:, qcc * 512 : (qcc + 1) * 512],
                        start=(t == 0),
                        stop=(t == MT - 1),
                    )
                osb = rot.tile([128, 512], F32, tag="osb", bufs=3, name="osb")
                nc.vector.tensor_copy(osb, ps[:, 0:512])
                qabs = qcp * 1024 + qcc * 512
                nc.sync.dma_start(
                    out=otd.ap()[m * 128 : (m + 1) * 128, qabs : qabs + 512],
                    in_=osb,
                )

            # ---------- attention pipeline ----------
            pend = deque()  # global: (t, qcp, kt, [pt_h0, pt_h1])
            live = {}  # (t, qcp) -> [ps_o_h0, ps_o_h1]

            def finish(t, qcp):
                """Drain ps_o to SBUF (frees PSUM fast), then normalize via a
                wide reciprocal computed through a DRAM transpose round-trip."""
                ps_o = live.pop((t, qcp))
                for hh in range(2):
                    ou = rot.tile(
                        [VW, 1024], F32, tag="ou", bufs=CFG["ou_bufs"], name="ou"
                    )
                    nc.vector.tensor_copy(ou, ps_o[hh])  # releases pso banks
                    sc = dsc.tile([1, 1024], F32, tag="sc", name="sc")
                    nc.sync.dma_start(out=sc, in_=ou[D : D + 1, :])
                    tr = rot.tile([128, 8], F32, tag="tr", bufs=4, name="tr")
                    nc.sync.dma_start(
                        out=tr, in_=sc[0, :].rearrange("(p o) -> p o", o=8)
                    )
                    trr = rot.tile([128, 8], F32, tag="trr", bufs=4, name="trr")
                    nc.vector.reciprocal(trr, tr)
                    sc2 = dsc.tile([1, 1024], F32, tag="sc2", name="sc2")
                    nc.sync.dma_start(
                        out=sc2[0, :].rearrange("(p o) -> p o", o=8), in_=trr
                    )
                    bc = rot.tile(
                        [64, 1024], F32, tag="bc", bufs=CFG["bc_bufs"], name="bc"
                    )
                    nc.sync.dma_start(out=bc, in_=sc2[0, :].partition_broadcast(64))
                    if hh == 0:
                        nc.vector.tensor_tensor(
                            oT[t][qcp][0:64, :], ou[0:D, :], bc, OP.mult
                        )
                    else:
                        otn = rot.tile([64, 1024], F16, tag="otn", bufs=2, name="otn")
                        nc.vector.tensor_tensor(otn, ou[0:D, :], bc, OP.mult)
                        nc.sync.dma_start(out=oT[t][qcp][64:128, :], in_=otn)

            def emit_av(t, qcp, kt, pts):
                # pts is indexed by qch; head hh's scores live in its
                # 512-column half of each qch tile.
                ps_o = live[(t, qcp)]
                for hh in range(2):
                    vb = (2 * t + hh) * VW
                    for qch in range(2):
                        nc.tensor.matmul(
                            ps_o[hh][:, qch * 512 : (qch + 1) * 512],
                            lhsT=vaug[kt][:, vb : vb + VW],
                            rhs=pts[qch][:, hh * 512 : (hh + 1) * 512],
                            start=(kt == 0),
                            stop=(kt == SQ - 1),
                        )
                if kt == SQ - 1:
                    finish(t, qcp)

            def attn_step(t, qcp, kt):
                """QK + exp for one kt of call (t, qcp); queue its AV. The two
                heads' QK matmuls are emitted adjacently per qch so their
                row-group tiles (base partition 0 / 64) run concurrently."""
                if kt == 0:
                    live[(t, qcp)] = [
                        pso.tile([VW, 1024], F32, tag="o", name=f"pso{hh}")
                        for hh in range(2)
                    ]
                kqc, kof = kt // 4, (kt % 4) * 128
                # psum tiles split by q-chunk, NOT by head: both heads' QK
                # matmuls for one qch share a tile, so they become ready
                # together and issue back-to-back -> their row-group tiles
                # (base partition 0 / 64) execute concurrently in the array.
                ps_q = [
                    psmm.tile([128, 1024], F32, tag="mm", name="pss")
                    for qch in range(2)
                ]
                for qch in range(2):
                    qc = 2 * qcp + qch
                    for hh in range(2):
                        hp = 64 * hh
                        nc.tensor.matmul(
                            ps_q[qch][:, hh * 512 : (hh + 1) * 512],
                            lhsT=khT[t][kqc][hp : hp + 64, kof : kof + 128],
                            rhs=qhT[t][qc][hp : hp + 64, :],
                            start=True,
                            stop=True,
                        )
                pts = []
                for qch in range(2):
                    pt_t = rot.tile(
                        [128, 1024], F16, tag="pt", bufs=CFG["pt_bufs"], name="pt"
                    )
                    nc.scalar.activation(pt_t, ps_q[qch], AF.Exp, scale=SCALE)
                    pts.append(pt_t)
                pend.append((t, qcp, kt, pts))
                while len(pend) > CFG["av_depth"]:
                    emit_av(*pend.popleft())

            def flush_pend():
                while pend:
                    emit_av(*pend.popleft())

            # ---------- global emission schedule: deadline weave ----------
            # Calls in order c = qcp*4 + t; global step s = c*16 + kt.
            # Background units carry the step index that first consumes them;
            # each is emitted just before that step (plus a small drip-ahead).
            AVD = CFG["av_depth"]
            PREF = CFG["prefetch_ahead"]
            P = [proj_units(qc) for qc in range(4)]  # [v0..3, k0..3, q0..3]
            units = []  # (deadline, seq, burst, prefetch)
            seq = 0

            def add_unit(dl, pair):
                nonlocal seq
                pf, th = pair if isinstance(pair, tuple) else (None, pair)
                units.append((dl, seq, th, pf))
                seq += 1

            for qc in range(4):
                vs, ks, qs = P[qc][0:4], P[qc][4:8], P[qc][8:12]
                for sql in range(4):
                    kb = qc * 4 + sql
                    add_unit(kb + AVD, vs[sql])  # consumed by AV(c0, kb)
                for m in range(4):
                    add_unit(m * 16 + 4 * qc, ks[m])  # QK(call m, kt=4qc)
                qcp = qc // 2
                for m in range(4):
                    dl = (qcp * 4 + m) * 16
                    add_unit(max(0, dl - 4 + 2 * (qc % 2)), qs[m])
            add_unit(56, stage_wo)  # wo needed by o_bursts (from step ~68)
            units.sort(key=lambda u: (u[0], u[1]))
            units = deque(units)
            pf_queue = deque(units)  # same order; prefetches run PREF early

            obg = deque((0, qcc, m) for qcc in range(2) for m in range(H // 128))

            calls = [(t, qcp) for qcp in range(2) for t in range(MT)]
            for c, (t, qcp) in enumerate(calls):
                for kt in range(SQ):
                    s = c * 16 + kt
                    while pf_queue and pf_queue[0][0] <= s + PREF:
                        u = pf_queue.popleft()
                        if u[3] is not None:
                            u[3]()
                    while units and units[0][0] <= s:
                        units.popleft()[2]()
                    attn_step(t, qcp, kt)
                    # drip-ahead: pull at most one near-future unit per step
                    if units and units[0][0] <= s + 4:
                        units.popleft()[2]()
                    # out_proj(qcp0) once all its normalizes are emitted
                    # (finish(3,0) pops at step 67)
                    if s >= 68 and obg and s % 3 == 0:
                        o_burst(*obg.popleft())
                    # last call: ramp the AV queue down so the final
                    # normalize (the out_proj(qcp1) gate) starts ASAP
                    if c == 7 and kt >= 11:
                        while len(pend) > 15 - kt:
                            emit_av(*pend.popleft())
            flush_pend()  # drains last AVs + final normalizes

            # tail: any remaining out_proj
            for u in pf_queue:
                if u[3] is not None:
                    u[3]()
            while units:
                units.popleft()[2]()
            while obg:
                o_burst(*obg.popleft())
            for qcc in range(2):
                for m in range(H // 128):
                    o_burst(1, qcc, m)

    nc.compile()
    return nc


def _get_nc():
    if "nc" not in _CACHE:
        _CACHE["nc"] = _build()
    return _CACHE["nc"]


def make_in_maps(q, k, v, wq, wk, wv, wo, bq):
    q = np.asarray(q, np.float32)
    k = np.asarray(k, np.float32)
    v = np.asarray(v, np.float32)
    in_maps = []
    for c in range(NCORES):
        g, b = divmod(c, B)
        sl = slice(g * GH, (g + 1) * GH)
        in_maps.append(
            {
                "xq": np.ascontiguousarray(q[b].T).astype(np.float16),
                "xk": np.ascontiguousarray(k[b].T).astype(np.float16),
                "xv": np.ascontiguousarray(v[b].T).astype(np.float16),
                "wq": np.ascontiguousarray(np.asarray(wq, np.float32)[:, sl]).astype(
                    np.float16
                ),
                "wk": np.ascontiguousarray(np.asarray(wk, np.float32)[:, sl]).astype(
                    np.float16
                ),
                "wv": np.ascontiguousarray(np.asarray(wv, np.float32)[:, sl]).astype(
                    np.float16
                ),
                "wo": np.ascontiguousarray(np.asarray(wo, np.float32)[sl, :]),
                "bq": np.ascontiguousarray(np.asarray(bq, np.float32)[sl]),
            }
        )
    return in_maps


def assemble(per_core_ot, bv, wo, bo):
    bo_eff = (
        np.asarray(bo, np.float32)
        + np.asarray(bv, np.float32) @ np.asarray(wo, np.float32)
    )
    out = np.empty((B, S, H), np.float32)
    for b in range(B):
        out[b] = per_core_ot[b].T + per_core_ot[B + b].T + bo_eff
    return out


def kernel(q, k, v, wq, bq, wk, bk, wv, bv, wo, bo, _trace=False):
    from concourse.bass_utils import run_bass_kernel_spmd

    nc = _get_nc()
    in_maps = make_in_maps(q, k, v, wq, wk, wv, wo, bq)
    res = run_bass_kernel_spmd(
        nc, in_maps, core_ids=list(range(NCORES)), trace=_trace
    )
    _CACHE["last_results"] = res
    outs = [res.results[c]["ot"] for c in range(NCORES)]
    return assemble(outs, bv, wo, bo)


# revision 17
# speedup vs baseline: 1.0056x; 1.0056x over previous
"""Multi-head attention (B=4,S=2048,H=1024,NH=16,D=64) on 8 trn2 cores.

Sharding: core c = (g, b) with g = c // 4 (head-group of 8 heads = 512 dims,
tensor parallel) and b = c % 4 (batch, data parallel). Each core computes a
partial output (its head-group's contribution to the final projection),
transposed: ot = (attn_out_g @ wo_g)^T of shape [H, S]. Host sums the two
group partials per batch and adds bias.

Math notes (host/device split):
  - k-proj bias bk drops out of softmax (adds a per-query constant along the
    key axis), so it is not applied on device.
  - v-proj bias bv commutes through normalized attention (rows of the score
    matrix sum to 1): its contribution is bv @ wo, folded into the output
    bias on the host.

On-device layout: everything is computed transposed (feature dim on
partitions, sequence on the free axis) so the softmax key-axis lands on
partitions. Scores S^T are built per head as K_h^T(stationary) x Q_h^T,
exp() runs on the scalar engine straight out of PSUM, and the ones-column
appended to V in the AV matmul yields the softmax denominators for free.

Schedule: the kernel is ACT-bound (256 exp instructions ~ 293us floor);
emission is one global pipeline: projections are split into per-qc units
interleaved with the first attention call's kt-steps (ramp), all attention
(t, qcp) calls share one global pending-AV queue so call boundaries stitch
without ACT gaps, and softmax normalization drains PSUM immediately (DVE
copy) then computes the reciprocal wide ([128,8] via a DRAM transpose
round-trip) off the critical path.
"""

import sys

if "/opt/trn_rl_repo" not in sys.path:
    sys.path.insert(0, "/opt/trn_rl_repo")

from collections import deque

import numpy as np

B, S, H, NH, D = 4, 2048, 1024, 16, 64
G = 2  # head-group split across cores (tensor parallel axis)
GH = H // G  # 512 dims (8 heads) per group
NCORES = 8
SCALE = 1.0 / float(D) ** 0.5  # 1/8

KT = H // 128  # 8 contraction tiles for projections
MT = GH // 128  # 4 m-tiles = head pairs per group
NQC = S // 512  # 4 sequence chunks of 512
SQ = S // 128  # 16 key-sequence tiles
VW = D + 1  # 65: V columns + ones column per head

_CACHE = {}

CFG = {
    "xs_bufs": 32,
    "pt_bufs": 12,
    "av_depth": 3,  # pending-AV queue depth (global, in kt-steps)
    "prefetch_ahead": 6,  # strip-DMA prefetch distance, in kt-steps
    "ou_bufs": 3,
    "bc_bufs": 2,
}


def _build():
    import concourse.tile as tile
    from concourse import bacc, mybir

    F32 = mybir.dt.float32
    F16 = mybir.dt.float16
    AF = mybir.ActivationFunctionType
    OP = mybir.AluOpType

    nc = bacc.Bacc("TRN2", target_bir_lowering=False, debug=False)

    xq = nc.dram_tensor("xq", [H, S], F16, kind="ExternalInput")
    xk = nc.dram_tensor("xk", [H, S], F16, kind="ExternalInput")
    xv = nc.dram_tensor("xv", [H, S], F16, kind="ExternalInput")
    wqd = nc.dram_tensor("wq", [H, GH], F16, kind="ExternalInput")
    wkd = nc.dram_tensor("wk", [H, GH], F16, kind="ExternalInput")
    wvd = nc.dram_tensor("wv", [H, GH], F16, kind="ExternalInput")
    wod = nc.dram_tensor("wo", [GH, H], F32, kind="ExternalInput")
    bqd = nc.dram_tensor("bq", [GH], F32, kind="ExternalInput")
    otd = nc.dram_tensor("ot", [H, S], F32, kind="ExternalOutput")

    with tile.TileContext(nc) as tc:
        with (
            tc.tile_pool(name="res", bufs=1) as res,
            tc.tile_pool(name="rot", bufs=2) as rot,
            tc.tile_pool(name="psmm", bufs=2, space="PSUM") as psmm,
            tc.tile_pool(name="pso", bufs=2, space="PSUM") as pso,
            tc.tile_pool(name="dsc", bufs=8, space="DRAM") as dsc,
        ):
            # ---- residents (fine-grained for emission-time dep tracking) ----
            qhT = [
                [
                    res.tile([128, 512], F16, tag=f"qhT{m}_{qc}", name=f"qhT{m}_{qc}")
                    for qc in range(4)
                ]
                for m in range(MT)
            ]
            khT = [
                [
                    res.tile([128, 512], F16, tag=f"khT{m}_{qc}", name=f"khT{m}_{qc}")
                    for qc in range(4)
                ]
                for m in range(MT)
            ]
            oT = [
                [
                    res.tile([128, 1024], F16, tag=f"oT{t}_{qcp}", name=f"oT{t}_{qcp}")
                    for qcp in range(2)
                ]
                for t in range(MT)
            ]
            vaug = [
                res.tile([128, 8 * VW], F16, tag=f"vaug{kb}", name=f"vaug{kb}")
                for kb in range(SQ)
            ]
            wo_bf = [
                res.tile([128, H], F16, tag=f"wob{t}", name=f"wob{t}")
                for t in range(MT)
            ]
            wq_sb = [
                res.tile([128, GH], F16, tag=f"wq{kt}", name=f"wq{kt}")
                for kt in range(KT)
            ]
            wk_sb = [
                res.tile([128, GH], F16, tag=f"wk{kt}", name=f"wk{kt}")
                for kt in range(KT)
            ]
            wv_sb = [
                res.tile([128, GH], F16, tag=f"wv{kt}", name=f"wv{kt}")
                for kt in range(KT)
            ]
            bq_sb = res.tile([128, MT], F32, tag="bqsb", name="bq_sb")

            # ---- early staging, ordered by first use: the first bursts are
            # K(0,0)/Q(0,0)/Q(1,0), so wk/wq land first; wv follows (V bursts
            # start at step 4); wo staging is deferred into the weave.
            for kt in range(KT):
                nc.scalar.dma_start(
                    out=wk_sb[kt], in_=wkd.ap()[kt * 128 : (kt + 1) * 128, :]
                )
            for kt in range(KT):
                nc.scalar.dma_start(
                    out=wq_sb[kt], in_=wqd.ap()[kt * 128 : (kt + 1) * 128, :]
                )
            for m in range(MT):
                nc.scalar.dma_start(
                    out=bq_sb[:, m : m + 1],
                    in_=bqd.ap()[m * 128 : (m + 1) * 128].rearrange(
                        "(p o) -> p o", o=1
                    ),
                )
            for kt in range(KT):
                nc.scalar.dma_start(
                    out=wv_sb[kt], in_=wvd.ap()[kt * 128 : (kt + 1) * 128, :]
                )
            # ones columns of the augmented-V tiles (V slots overwritten later)
            for kb in range(SQ):
                nc.vector.memset(vaug[kb], 1.0)

            def stage_wo():
                for t in range(MT):
                    wos = rot.tile([128, H], F32, tag="wos", bufs=2, name=f"wos{t}")
                    nc.sync.dma_start(
                        out=wos, in_=wod.ap()[t * 128 : (t + 1) * 128, :]
                    )
                    nc.vector.tensor_copy(wo_bf[t], wos)

            def load_strips(xd, qc):
                xs = []
                for kt in range(KT):
                    st = rot.tile(
                        [128, 512], F16, tag="xs", bufs=CFG["xs_bufs"], name=f"xs{kt}"
                    )
                    nc.sync.dma_start(
                        out=st,
                        in_=xd.ap()[
                            kt * 128 : (kt + 1) * 128, qc * 512 : (qc + 1) * 512
                        ],
                    )
                    xs.append(st)
                return xs

            # ---------- projection unit emitters (one psum burst each) ----------
            def v_burst(xs, qc, sql):
                kb = qc * 4 + sql
                ps = psmm.tile([128, 1024], F32, tag="mm", name=f"psv{kb}")
                for kt in range(KT):
                    nc.tensor.matmul(
                        ps[:, 0:512],
                        lhsT=xs[kt][:, sql * 128 : (sql + 1) * 128],
                        rhs=wv_sb[kt],
                        start=(kt == 0),
                        stop=(kt == KT - 1),
                    )
                for h in range(8):
                    nc.vector.tensor_copy(
                        vaug[kb][:, h * VW : h * VW + D],
                        ps[:, h * D : (h + 1) * D],
                    )

            def k_burst(xs, qc, m):
                ps = psmm.tile([128, 1024], F32, tag="mm", name=f"psk{m}")
                for kt in range(KT):
                    nc.tensor.matmul(
                        ps[:, 0:512],
                        lhsT=wk_sb[kt][:, m * 128 : (m + 1) * 128],
                        rhs=xs[kt],
                        start=(kt == 0),
                        stop=(kt == KT - 1),
                    )
                nc.vector.tensor_copy(khT[m][qc], ps[:, 0:512])

            def q_burst(xs, qc, m):
                ps = psmm.tile([128, 1024], F32, tag="mm", name=f"psq{m}")
                for kt in range(KT):
                    nc.tensor.matmul(
                        ps[:, 0:512],
                        lhsT=wq_sb[kt][:, m * 128 : (m + 1) * 128],
                        rhs=xs[kt],
                        start=(kt == 0),
                        stop=(kt == KT - 1),
                    )
                nc.vector.tensor_scalar(
                    qhT[m][qc], ps[:, 0:512], bq_sb[:, m : m + 1], None, OP.add
                )

            def proj_units(qc):
                """The 12 projection bursts for one 512-seq chunk, as
                (prefetch, burst) thunk pairs. Each burst has its own strip
                load (bursts are woven far apart, so sharing strips across
                bursts would stretch xs-ring lifetimes into dependency
                cycles); the prefetch is emitted several steps ahead so the
                1MB strip DMA never sits on the QK critical path. V's four
                bursts are adjacent in the weave and share one load."""
                us = []
                vstate = {}

                def mk(kind, m):
                    slot = {}

                    def prefetch():
                        if kind == "v":
                            if "xs" not in vstate:
                                vstate["xs"] = load_strips(xv, qc)
                        else:
                            xd = xk if kind == "k" else xq
                            slot["xs"] = load_strips(xd, qc)

                    def run():
                        if kind == "v":
                            v_burst(vstate["xs"], qc, m)
                        elif kind == "k":
                            k_burst(slot["xs"], qc, m)
                        else:
                            q_burst(slot["xs"], qc, m)

                    return prefetch, run

                for kind in ("v", "k", "q"):
                    for m in range(4):
                        us.append(mk(kind, m))
                return us

            # ---------- out-projection (one 128-row output block) ----------
            def o_burst(qcp, qcc, m):
                ps = psmm.tile([128, 1024], F32, tag="mm", name=f"pso{m}")
                for t in range(MT):
                    nc.tensor.matmul(
                        ps[:, 0:512],
                        lhsT=wo_bf[t][:, m * 128 : (m + 1) * 128],
                        rhs=oT[t][qcp][:, qcc * 512 : (qcc + 1) * 512],
                        start=(t == 0),
                        stop=(t == MT - 1),
                    )
                osb = rot.tile([128, 512], F32, tag="osb", bufs=3, name="osb")
                nc.vector.tensor_copy(osb, ps[:, 0:512])
                qabs = qcp * 1024 + qcc * 512
                nc.sync.dma_start(
                    out=otd.ap()[m * 128 : (m + 1) * 128, qabs : qabs + 512],
                    in_=osb,
                )

            # ---------- attention pipeline ----------
            pend = deque()  # global: (t, qcp, kt, [pt_h0, pt_h1])
            live = {}  # (t, qcp) -> [ps_o_h0, ps_o_h1]

            def finish(t, qcp):
                """Drain ps_o to SBUF (frees PSUM fast), then normalize via a
                wide reciprocal computed through a DRAM transpose round-trip."""
                ps_o = live.pop((t, qcp))
                for hh in range(2):
                    ou = rot.tile(
                        [VW, 1024], F32, tag="ou", bufs=CFG["ou_bufs"], name="ou"
                    )
                    nc.vector.tensor_copy(ou, ps_o[hh])  # releases pso banks
                    sc = dsc.tile([1, 1024], F32, tag="sc", name="sc")
                    nc.sync.dma_start(out=sc, in_=ou[D : D + 1, :])
                    tr = rot.tile([128, 8], F32, tag="tr", bufs=4, name="tr")
                    nc.sync.dma_start(
                        out=tr, in_=sc[0, :].rearrange("(p o) -> p o", o=8)
                    )
                    trr = rot.tile([128, 8], F32, tag="trr", bufs=4, name="trr")
                    nc.vector.reciprocal(trr, tr)
                    sc2 = dsc.tile([1, 1024], F32, tag="sc2", name="sc2")
                    nc.sync.dma_start(
                        out=sc2[0, :].rearrange("(p o) -> p o", o=8), in_=trr
                    )
                    bc = rot.tile(
                        [64, 1024], F32, tag="bc", bufs=CFG["bc_bufs"], name="bc"
                    )
                    nc.sync.dma_start(out=bc, in_=sc2[0, :].partition_broadcast(64))
                    if hh == 0:
                        nc.vector.tensor_tensor(
                            oT[t][qcp][0:64, :], ou[0:D, :], bc, OP.mult
                        )
                    else:
                        otn = rot.tile([64, 1024], F16, tag="otn", bufs=2, name="otn")
                        nc.vector.tensor_tensor(otn, ou[0:D, :], bc, OP.mult)
                        nc.sync.dma_start(out=oT[t][qcp][64:128, :], in_=otn)

            def emit_av(t, qcp, kt, pts):
                # pts is indexed by qch; head hh's scores live in its
                # 512-column half of each qch tile.
                ps_o = live[(t, qcp)]
                for hh in range(2):
                    vb = (2 * t + hh) * VW
                    for qch in range(2):
                        nc.tensor.matmul(
                            ps_o[hh][:, qch * 512 : (qch + 1) * 512],
                            lhsT=vaug[kt][:, vb : vb + VW],
                            rhs=pts[qch][:, hh * 512 : (hh + 1) * 512],
                            start=(kt == 0),
                            stop=(kt == SQ - 1),
                        )
                if kt == SQ - 1:
                    finish(t, qcp)

            def attn_step(t, qcp, kt):
                """QK + exp for one kt of call (t, qcp); queue its AV. The two
                heads' QK matmuls are emitted adjacently per qch so their
                row-group tiles (base partition 0 / 64) run concurrently."""
                if kt == 0:
                    live[(t, qcp)] = [
                        pso.tile([VW, 1024], F32, tag="o", name=f"pso{hh}")
                        for hh in range(2)
                    ]
                kqc, kof = kt // 4, (kt % 4) * 128
                # psum tiles split by q-chunk, NOT by head: both heads' QK
                # matmuls for one qch share a tile, so they become ready
                # together and issue back-to-back -> their row-group tiles
                # (base partition 0 / 64) execute concurrently in the array.
                ps_q = [
                    psmm.tile([128, 1024], F32, tag="mm", name="pss")
                    for qch in range(2)
                ]
                for qch in range(2):
                    qc = 2 * qcp + qch
                    for hh in range(2):
                        hp = 64 * hh
                        nc.tensor.matmul(
                            ps_q[qch][:, hh * 512 : (hh + 1) * 512],
                            lhsT=khT[t][kqc][hp : hp + 64, kof : kof + 128],
                            rhs=qhT[t][qc][hp : hp + 64, :],
                            start=True,
                            stop=True,
                        )
                pts = []
                for qch in range(2):
                    pt_t = rot.tile(
                        [128, 1024], F16, tag="pt", bufs=CFG["pt_bufs"], name="pt"
                    )
                    nc.scalar.activation(pt_t, ps_q[qch], AF.Exp, scale=SCALE)
                    pts.append(pt_t)
                pend.append((t, qcp, kt, pts))
                while len(pend) > CFG["av_depth"]:
                    emit_av(*pend.popleft())

            def flush_pend():
                while pend:
                    emit_av(*pend.popleft())

            # ---------- global emission schedule: deadline weave ----------
            # Calls in order c = qcp*4 + t; global step s = c*16 + kt.
            # Background units carry the step index that first consumes them;
            # each is emitted just before that step (plus a small drip-ahead).
            AVD = CFG["av_depth"]
            PREF = CFG["prefetch_ahead"]
            P = [proj_units(qc) for qc in range(4)]  # [v0..3, k0..3, q0..3]
            units = []  # (deadline, seq, burst, prefetch)
            seq = 0

            def add_unit(dl, pair):
                nonlocal seq
                pf, th = pair if isinstance(pair, tuple) else (None, pair)
                units.append((dl, seq, th, pf))
                seq += 1

            for qc in range(4):
                vs, ks, qs = P[qc][0:4], P[qc][4:8], P[qc][8:12]
                for sql in range(4):
                    kb = qc * 4 + sql
                    add_unit(kb + AVD, vs[sql])  # consumed by AV(c0, kb)
                for m in range(4):
                    add_unit(m * 16 + 4 * qc, ks[m])  # QK(call m, kt=4qc)
                qcp = qc // 2
                for m in range(4):
                    dl = (qcp * 4 + m) * 16
                    add_unit(max(0, dl - 4 + 2 * (qc % 2)), qs[m])
            add_unit(56, stage_wo)  # wo needed by o_bursts (from step ~68)
            units.sort(key=lambda u: (u[0], u[1]))
            units = deque(units)
            pf_queue = deque(units)  # same order; prefetches run PREF early

            obg = deque((0, qcc, m) for qcc in range(2) for m in range(H // 128))

            calls = [(t, qcp) for qcp in range(2) for t in range(MT)]
            for c, (t, qcp) in enumerate(calls):
                for kt in range(SQ):
                    s = c * 16 + kt
                    while pf_queue and pf_queue[0][0] <= s + PREF:
                        u = pf_queue.popleft()
                        if u[3] is not None:
                            u[3]()
                    while units and units[0][0] <= s:
                        units.popleft()[2]()
                    attn_step(t, qcp, kt)
                    # drip-ahead: pull at most one near-future unit per step
                    if units and units[0][0] <= s + 4:
                        units.popleft()[2]()
                    # out_proj(qcp0) once all its normalizes are emitted
                    # (finish(3,0) pops at step 67)
                    if s >= 68 and obg and s % 3 == 0:
                        o_burst(*obg.popleft())
                    # last call: ramp the AV queue down so the final
                    # normalize (the out_proj(qcp1) gate) starts ASAP
                    if c == 7 and kt >= 11:
                        while len(pend) > 15 - kt:
                            emit_av(*pend.popleft())
            flush_pend()  # drains last AVs + final normalizes

            # tail: any remaining out_proj
            for u in pf_queue:
                if u[3] is not None:
                    u[3]()
            while units:
                units.popleft()[2]()
            while obg:
                o_burst(*obg.popleft())
            for qcc in range(2):
                for m in range(H // 128):
                    o_burst(1, qcc, m)

    nc.compile()
    return nc


def _get_nc():
    if "nc" not in _CACHE:
        _CACHE["nc"] = _build()
    return _CACHE["nc"]


def make_in_maps(q, k, v, wq, wk, wv, wo, bq):
    q = np.asarray(q, np.float32)
    k = np.asarray(k, np.float32)
    v = np.asarray(v, np.float32)
    in_maps = []
    for c in range(NCORES):
        g, b = divmod(c, B)
        sl = slice(g * GH, (g + 1) * GH)
        in_maps.append(
            {
                "xq": np.ascontiguousarray(q[b].T).astype(np.float16),
                "xk": np.ascontiguousarray(k[b].T).astype(np.float16),
                "xv": np.ascontiguousarray(v[b].T).astype(np.float16),
                "wq": np.ascontiguousarray(np.asarray(wq, np.float32)[:, sl]).astype(
                    np.float16
                ),
                "wk": np.ascontiguousarray(np.asarray(wk, np.float32)[:, sl]).astype(
                    np.float16
                ),
                "wv": np.ascontiguousarray(np.asarray(wv, np.float32)[:, sl]).astype(
                    np.float16
                ),
                "wo": np.ascontiguousarray(np.asarray(wo, np.float32)[sl, :]),
                "bq": np.ascontiguousarray(np.asarray(bq, np.float32)[sl]),
            }
        )
    return in_maps


def assemble(per_core_ot, bv, wo, bo):
    bo_eff = (
        np.asarray(bo, np.float32)
        + np.asarray(bv, np.float32) @ np.asarray(wo, np.float32)
    )
    out = np.empty((B, S, H), np.float32)
    for b in range(B):
        out[b] = per_core_ot[b].T + per_core_ot[B + b].T + bo_eff
    return out


def kernel(q, k, v, wq, bq, wk, bk, wv, bv, wo, bo, _trace=False):
    from concourse.bass_utils import run_bass_kernel_spmd

    nc = _get_nc()
    in_maps = make_in_maps(q, k, v, wq, wk, wv, wo, bq)
    res = run_bass_kernel_spmd(
        nc, in_maps, core_ids=list(range(NCORES)), trace=_trace
    )
    _CACHE["last_results"] = res
    outs = [res.results[c]["ot"] for c in range(NCORES)]
    return assemble(outs, bv, wo, bo)


# revision 19
# speedup vs baseline: 1.0076x; 1.0020x over previous
"""Multi-head attention (B=4,S=2048,H=1024,NH=16,D=64) on 8 trn2 cores.

Sharding: core c = (g, b) with g = c // 4 (head-group of 8 heads = 512 dims,
tensor parallel) and b = c % 4 (batch, data parallel). Each core computes a
partial output (its head-group's contribution to the final projection),
transposed: ot = (attn_out_g @ wo_g)^T of shape [H, S]. Host sums the two
group partials per batch and adds bias.

Math notes (host/device split):
  - k-proj bias bk drops out of softmax (adds a per-query constant along the
    key axis), so it is not applied on device.
  - v-proj bias bv commutes through normalized attention (rows of the score
    matrix sum to 1): its contribution is bv @ wo, folded into the output
    bias on the host.

On-device layout: everything is computed transposed (feature dim on
partitions, sequence on the free axis) so the softmax key-axis lands on
partitions. Scores S^T are built per head as K_h^T(stationary) x Q_h^T,
exp() runs on the scalar engine straight out of PSUM, and the ones-column
appended to V in the AV matmul yields the softmax denominators for free.

Schedule: the kernel is ACT-bound (256 exp instructions ~ 293us floor);
emission is one global pipeline: projections are split into per-qc units
interleaved with the first attention call's kt-steps (ramp), all attention
(t, qcp) calls share one global pending-AV queue so call boundaries stitch
without ACT gaps, and softmax normalization drains PSUM immediately (DVE
copy) then computes the reciprocal wide ([128,8] via a DRAM transpose
round-trip) off the critical path.
"""

import sys

if "/opt/trn_rl_repo" not in sys.path:
    sys.path.insert(0, "/opt/trn_rl_repo")

from collections import deque

import numpy as np

B, S, H, NH, D = 4, 2048, 1024, 16, 64
G = 2  # head-group split across cores (tensor parallel axis)
GH = H // G  # 512 dims (8 heads) per group
NCORES = 8
SCALE = 1.0 / float(D) ** 0.5  # 1/8

KT = H // 128  # 8 contraction tiles for projections
MT = GH // 128  # 4 m-tiles = head pairs per group
NQC = S // 512  # 4 sequence chunks of 512
SQ = S // 128  # 16 key-sequence tiles
VW = D + 1  # 65: V columns + ones column per head

_CACHE = {}

CFG = {
    "xs_bufs": 32,
    "pt_bufs": 14,
    "av_depth": 3,  # pending-AV queue depth (global, in kt-steps)
    "prefetch_ahead": 6,  # strip-DMA prefetch distance, in kt-steps
    "ou_bufs": 3,
    "bc_bufs": 3,
}


def _build():
    import concourse.tile as tile
    from concourse import bacc, mybir

    F32 = mybir.dt.float32
    F16 = mybir.dt.float16
    AF = mybir.ActivationFunctionType
    OP = mybir.AluOpType

    nc = bacc.Bacc("TRN2", target_bir_lowering=False, debug=False)

    xq = nc.dram_tensor("xq", [H, S], F16, kind="ExternalInput")
    xk = nc.dram_tensor("xk", [H, S], F16, kind="ExternalInput")
    xv = nc.dram_tensor("xv", [H, S], F16, kind="ExternalInput")
    wqd = nc.dram_tensor("wq", [H, GH], F16, kind="ExternalInput")
    wkd = nc.dram_tensor("wk", [H, GH], F16, kind="ExternalInput")
    wvd = nc.dram_tensor("wv", [H, GH], F16, kind="ExternalInput")
    wod = nc.dram_tensor("wo", [GH, H], F32, kind="ExternalInput")
    bqd = nc.dram_tensor("bq", [GH], F32, kind="ExternalInput")
    otd = nc.dram_tensor("ot", [H, S], F32, kind="ExternalOutput")

    with tile.TileContext(nc) as tc:
        with (
            tc.tile_pool(name="res", bufs=1) as res,
            tc.tile_pool(name="rot", bufs=2) as rot,
            tc.tile_pool(name="psmm", bufs=2, space="PSUM") as psmm,
            tc.tile_pool(name="pso", bufs=2, space="PSUM") as pso,
            tc.tile_pool(name="dsc", bufs=8, space="DRAM") as dsc,
        ):
            # ---- residents (fine-grained for emission-time dep tracking) ----
            qhT = [
                [
                    res.tile([128, 512], F16, tag=f"qhT{m}_{qc}", name=f"qhT{m}_{qc}")
                    for qc in range(4)
                ]
                for m in range(MT)
            ]
            khT = [
                [
                    res.tile([128, 512], F16, tag=f"khT{m}_{qc}", name=f"khT{m}_{qc}")
                    for qc in range(4)
                ]
                for m in range(MT)
            ]
            oT = [
                [
                    res.tile([128, 1024], F16, tag=f"oT{t}_{qcp}", name=f"oT{t}_{qcp}")
                    for qcp in range(2)
                ]
                for t in range(MT)
            ]
            vaug = [
                res.tile([128, 8 * VW], F16, tag=f"vaug{kb}", name=f"vaug{kb}")
                for kb in range(SQ)
            ]
            wo_bf = [
                res.tile([128, H], F16, tag=f"wob{t}", name=f"wob{t}")
                for t in range(MT)
            ]
            wq_sb = [
                res.tile([128, GH], F16, tag=f"wq{kt}", name=f"wq{kt}")
                for kt in range(KT)
            ]
            wk_sb = [
                res.tile([128, GH], F16, tag=f"wk{kt}", name=f"wk{kt}")
                for kt in range(KT)
            ]
            wv_sb = [
                res.tile([128, GH], F16, tag=f"wv{kt}", name=f"wv{kt}")
                for kt in range(KT)
            ]
            bq_sb = res.tile([128, MT], F32, tag="bqsb", name="bq_sb")

            # ---- early staging, ordered by first use: the first bursts are
            # K(0,0)/Q(0,0)/Q(1,0), so wk/wq land first; wv follows (V bursts
            # start at step 4); wo staging is deferred into the weave.
            for kt in range(KT):
                nc.scalar.dma_start(
                    out=wk_sb[kt], in_=wkd.ap()[kt * 128 : (kt + 1) * 128, :]
                )
            for kt in range(KT):
                nc.scalar.dma_start(
                    out=wq_sb[kt], in_=wqd.ap()[kt * 128 : (kt + 1) * 128, :]
                )
            for m in range(MT):
                nc.scalar.dma_start(
                    out=bq_sb[:, m : m + 1],
                    in_=bqd.ap()[m * 128 : (m + 1) * 128].rearrange(
                        "(p o) -> p o", o=1
                    ),
                )
            for kt in range(KT):
                nc.scalar.dma_start(
                    out=wv_sb[kt], in_=wvd.ap()[kt * 128 : (kt + 1) * 128, :]
                )
            # ones columns of the augmented-V tiles (V slots overwritten later)
            for kb in range(SQ):
                nc.vector.memset(vaug[kb], 1.0)

            def stage_wo():
                for t in range(MT):
                    wos = rot.tile([128, H], F32, tag="wos", bufs=2, name=f"wos{t}")
                    nc.sync.dma_start(
                        out=wos, in_=wod.ap()[t * 128 : (t + 1) * 128, :]
                    )
                    nc.vector.tensor_copy(wo_bf[t], wos)

            def load_strips(xd, qc, eng=None):
                eng = eng or nc.sync
                xs = []
                for kt in range(KT):
                    st = rot.tile(
                        [128, 512], F16, tag="xs", bufs=CFG["xs_bufs"], name=f"xs{kt}"
                    )
                    eng.dma_start(
                        out=st,
                        in_=xd.ap()[
                            kt * 128 : (kt + 1) * 128, qc * 512 : (qc + 1) * 512
                        ],
                    )
                    xs.append(st)
                return xs

            # ---------- projection unit emitters (one psum burst each) ----------
            def v_burst(xs, qc, sql):
                kb = qc * 4 + sql
                ps = psmm.tile([128, 1024], F32, tag="mm", name=f"psv{kb}")
                for kt in range(KT):
                    nc.tensor.matmul(
                        ps[:, 0:512],
                        lhsT=xs[kt][:, sql * 128 : (sql + 1) * 128],
                        rhs=wv_sb[kt],
                        start=(kt == 0),
                        stop=(kt == KT - 1),
                    )
                for h in range(8):
                    nc.vector.tensor_copy(
                        vaug[kb][:, h * VW : h * VW + D],
                        ps[:, h * D : (h + 1) * D],
                    )

            def k_burst(xs, qc, m):
                ps = psmm.tile([128, 1024], F32, tag="mm", name=f"psk{m}")
                for kt in range(KT):
                    nc.tensor.matmul(
                        ps[:, 0:512],
                        lhsT=wk_sb[kt][:, m * 128 : (m + 1) * 128],
                        rhs=xs[kt],
                        start=(kt == 0),
                        stop=(kt == KT - 1),
                    )
                nc.vector.tensor_copy(khT[m][qc], ps[:, 0:512])

            def q_burst(xs, qc, m):
                ps = psmm.tile([128, 1024], F32, tag="mm", name=f"psq{m}")
                for kt in range(KT):
                    nc.tensor.matmul(
                        ps[:, 0:512],
                        lhsT=wq_sb[kt][:, m * 128 : (m + 1) * 128],
                        rhs=xs[kt],
                        start=(kt == 0),
                        stop=(kt == KT - 1),
                    )
                nc.vector.tensor_scalar(
                    qhT[m][qc], ps[:, 0:512], bq_sb[:, m : m + 1], None, OP.add
                )

            def proj_units(qc):
                """The 12 projection bursts for one 512-seq chunk, as
                (prefetch, burst) thunk pairs. Each burst has its own strip
                load (bursts are woven far apart, so sharing strips across
                bursts would stretch xs-ring lifetimes into dependency
                cycles); the prefetch is emitted several steps ahead so the
                1MB strip DMA never sits on the QK critical path. V's four
                bursts are adjacent in the weave and share one load."""
                us = []
                vstate = {}

                def mk(kind, m):
                    slot = {}

                    def prefetch():
                        if kind == "v":
                            if "xs" not in vstate:
                                vstate["xs"] = load_strips(xv, qc)
                        else:
                            xd = xk if kind == "k" else xq
                            slot["xs"] = load_strips(xd, qc)

                    def run():
                        if kind == "v":
                            v_burst(vstate["xs"], qc, m)
                        elif kind == "k":
                            k_burst(slot["xs"], qc, m)
                        else:
                            q_burst(slot["xs"], qc, m)

                    return prefetch, run

                for kind in ("v", "k", "q"):
                    for m in range(4):
                        us.append(mk(kind, m))
                return us

            # ---------- out-projection (one 128-row output block) ----------
            def o_burst(qcp, qcc, m):
                ps = psmm.tile([128, 1024], F32, tag="mm", name=f"pso{m}")
                for t in range(MT):
                    nc.tensor.matmul(
                        ps[:, 0:512],
                        lhsT=wo_bf[t][:, m * 128 : (m + 1) * 128],
                        rhs=oT[t][qcp][:, qcc * 512 : (qcc + 1) * 512],
                        start=(t == 0),
                        stop=(t == MT - 1),
                    )
                osb = rot.tile([128, 512], F32, tag="osb", bufs=3, name="osb")
                nc.vector.tensor_copy(osb, ps[:, 0:512])
                qabs = qcp * 1024 + qcc * 512
                nc.sync.dma_start(
                    out=otd.ap()[m * 128 : (m + 1) * 128, qabs : qabs + 512],
                    in_=osb,
                )

            # ---------- attention pipeline ----------
            pend = deque()  # global: (t, qcp, kt, [pt_h0, pt_h1])
            live = {}  # (t, qcp) -> [ps_o_h0, ps_o_h1]

            def finish(t, qcp):
                """Drain ps_o to SBUF (frees PSUM fast), then normalize via a
                wide reciprocal computed through a DRAM transpose round-trip."""
                ps_o = live.pop((t, qcp))
                for hh in range(2):
                    ou = rot.tile(
                        [VW, 1024], F32, tag="ou", bufs=CFG["ou_bufs"], name="ou"
                    )
                    nc.vector.tensor_copy(ou, ps_o[hh])  # releases pso banks
                    sc = dsc.tile([1, 1024], F32, tag="sc", name="sc")
                    nc.sync.dma_start(out=sc, in_=ou[D : D + 1, :])
                    tr = rot.tile([128, 8], F32, tag="tr", bufs=4, name="tr")
                    nc.sync.dma_start(
                        out=tr, in_=sc[0, :].rearrange("(p o) -> p o", o=8)
                    )
                    trr = rot.tile([128, 8], F32, tag="trr", bufs=4, name="trr")
                    nc.vector.reciprocal(trr, tr)
                    sc2 = dsc.tile([1, 1024], F32, tag="sc2", name="sc2")
                    nc.sync.dma_start(
                        out=sc2[0, :].rearrange("(p o) -> p o", o=8), in_=trr
                    )
                    bc = rot.tile(
                        [64, 1024], F32, tag="bc", bufs=CFG["bc_bufs"], name="bc"
                    )
                    nc.sync.dma_start(out=bc, in_=sc2[0, :].partition_broadcast(64))
                    if hh == 0:
                        nc.vector.tensor_tensor(
                            oT[t][qcp][0:64, :], ou[0:D, :], bc, OP.mult
                        )
                    else:
                        otn = rot.tile([64, 1024], F16, tag="otn", bufs=2, name="otn")
                        nc.vector.tensor_tensor(otn, ou[0:D, :], bc, OP.mult)
                        nc.sync.dma_start(out=oT[t][qcp][64:128, :], in_=otn)

            def emit_av(t, qcp, kt, pts):
                # pts is indexed by qch; head hh's scores live in its
                # 512-column half of each qch tile.
                ps_o = live[(t, qcp)]
                for hh in range(2):
                    vb = (2 * t + hh) * VW
                    for qch in range(2):
                        nc.tensor.matmul(
                            ps_o[hh][:, qch * 512 : (qch + 1) * 512],
                            lhsT=vaug[kt][:, vb : vb + VW],
                            rhs=pts[qch][:, hh * 512 : (hh + 1) * 512],
                            start=(kt == 0),
                            stop=(kt == SQ - 1),
                        )
                if kt == SQ - 1:
                    finish(t, qcp)

            def attn_step(t, qcp, kt):
                """QK + exp for one kt of call (t, qcp); queue its AV. The two
                heads' QK matmuls are emitted adjacently per qch so their
                row-group tiles (base partition 0 / 64) run concurrently."""
                if kt == 0:
                    live[(t, qcp)] = [
                        pso.tile([VW, 1024], F32, tag="o", name=f"pso{hh}")
                        for hh in range(2)
                    ]
                kqc, kof = kt // 4, (kt % 4) * 128
                # psum tiles split by q-chunk, NOT by head: both heads' QK
                # matmuls for one qch share a tile, so they become ready
                # together and issue back-to-back -> their row-group tiles
                # (base partition 0 / 64) execute concurrently in the array.
                ps_q = [
                    psmm.tile([128, 1024], F32, tag="mm", name="pss")
                    for qch in range(2)
                ]
                for qch in range(2):
                    qc = 2 * qcp + qch
                    for hh in range(2):
                        hp = 64 * hh
                        nc.tensor.matmul(
                            ps_q[qch][:, hh * 512 : (hh + 1) * 512],
                            lhsT=khT[t][kqc][hp : hp + 64, kof : kof + 128],
                            rhs=qhT[t][qc][hp : hp + 64, :],
                            start=True,
                            stop=True,
                        )
                pts = []
                for qch in range(2):
                    pt_t = rot.tile(
                        [128, 1024], F16, tag="pt", bufs=CFG["pt_bufs"], name="pt"
                    )
                    nc.scalar.activation(pt_t, ps_q[qch], AF.Exp, scale=SCALE)
                    pts.append(pt_t)
                pend.append((t, qcp, kt, pts))
                while len(pend) > CFG["av_depth"]:
                    emit_av(*pend.popleft())

            def flush_pend():
                while pend:
                    emit_av(*pend.popleft())

            # ---------- global emission schedule: deadline weave ----------
            # Calls in order c = qcp*4 + t; global step s = c*16 + kt.
            # Background units carry the step index that first consumes them;
            # each is emitted just before that step (plus a small drip-ahead).
            AVD = CFG["av_depth"]
            PREF = CFG["prefetch_ahead"]
            P = [proj_units(qc) for qc in range(4)]  # [v0..3, k0..3, q0..3]
            units = []  # (deadline, seq, burst, prefetch)
            seq = 0

            def add_unit(dl, pair):
                nonlocal seq
                pf, th = pair if isinstance(pair, tuple) else (None, pair)
                units.append((dl, seq, th, pf))
                seq += 1

            for qc in range(4):
                vs, ks, qs = P[qc][0:4], P[qc][4:8], P[qc][8:12]
                for sql in range(4):
                    kb = qc * 4 + sql
                    add_unit(kb + AVD, vs[sql])  # consumed by AV(c0, kb)
                for m in range(4):
                    add_unit(m * 16 + 4 * qc, ks[m])  # QK(call m, kt=4qc)
                qcp = qc // 2
                for m in range(4):
                    dl = (qcp * 4 + m) * 16
                    add_unit(max(0, dl - 4 + 2 * (qc % 2)), qs[m])
            add_unit(56, stage_wo)  # wo needed by o_bursts (from step ~68)
            units.sort(key=lambda u: (u[0], u[1]))
            units = deque(units)
            pf_queue = deque(units)  # same order; prefetches run PREF early

            obg = deque((0, qcc, m) for qcc in range(2) for m in range(H // 128))

            calls = [(t, qcp) for qcp in range(2) for t in range(MT)]
            for c, (t, qcp) in enumerate(calls):
                for kt in range(SQ):
                    s = c * 16 + kt
                    while pf_queue and pf_queue[0][0] <= s + PREF:
                        u = pf_queue.popleft()
                        if u[3] is not None:
                            u[3]()
                    while units and units[0][0] <= s:
                        units.popleft()[2]()
                    attn_step(t, qcp, kt)
                    # drip-ahead: pull at most one near-future unit per step
                    if units and units[0][0] <= s + 4:
                        units.popleft()[2]()
                    # out_proj(qcp0) once all its normalizes are emitted
                    # (finish(3,0) pops at step 67)
                    if s >= 68 and obg and s % 3 == 0:
                        o_burst(*obg.popleft())
                    # last call: ramp the AV queue down so the final
                    # normalize (the out_proj(qcp1) gate) starts ASAP
                    if c == 7 and kt >= 11:
                        while len(pend) > 15 - kt:
                            emit_av(*pend.popleft())
            flush_pend()  # drains last AVs + final normalizes

            # tail: any remaining out_proj
            for u in pf_queue:
                if u[3] is not None:
                    u[3]()
            while units:
                units.popleft()[2]()
            while obg:
                o_burst(*obg.popleft())
            for qcc in range(2):
                for m in range(H // 128):
                    o_burst(1, qcc, m)

    nc.compile()
    return nc


def _get_nc():
    if "nc" not in _CACHE:
        _CACHE["nc"] = _build()
    return _CACHE["nc"]


def make_in_maps(q, k, v, wq, wk, wv, wo, bq):
    q = np.asarray(q, np.float32)
    k = np.asarray(k, np.float32)
    v = np.asarray(v, np.float32)
    in_maps = []
    for c in range(NCORES):
        g, b = divmod(c, B)
        sl = slice(g * GH, (g + 1) * GH)
        in_maps.append(
            {
                "xq": np.ascontiguousarray(q[b].T).astype(np.float16),
                "xk": np.ascontiguousarray(k[b].T).astype(np.float16),
                "xv": np.ascontiguousarray(v[b].T).astype(np.float16),
                "wq": np.ascontiguousarray(np.asarray(wq, np.float32)[:, sl]).astype(
                    np.float16
                ),
                "wk": np.ascontiguousarray(np.asarray(wk, np.float32)[:, sl]).astype(
                    np.float16
                ),
                "wv": np.ascontiguousarray(np.asarray(wv, np.float32)[:, sl]).astype(
                    np.float16
                ),
                "wo": np.ascontiguousarray(np.asarray(wo, np.float32)[sl, :]),
                "bq": np.ascontiguousarray(np.asarray(bq, np.float32)[sl]),
            }
        )
    return in_maps


def assemble(per_core_ot, bv, wo, bo):
    bo_eff = (
        np.asarray(bo, np.float32)
        + np.asarray(bv, np.float32) @ np.asarray(wo, np.float32)
    )
    out = np.empty((B, S, H), np.float32)
    for b in range(B):
        out[b] = per_core_ot[b].T + per_core_ot[B + b].T + bo_eff
    return out


def kernel(q, k, v, wq, bq, wk, bk, wv, bv, wo, bo, _trace=False):
    from concourse.bass_utils import run_bass_kernel_spmd

    nc = _get_nc()
    in_maps = make_in_maps(q, k, v, wq, wk, wv, wo, bq)
    res = run_bass_kernel_spmd(
        nc, in_maps, core_ids=list(range(NCORES)), trace=_trace
    )
    _CACHE["last_results"] = res
    outs = [res.results[c]["ot"] for c in range(NCORES)]
    return assemble(outs, bv, wo, bo)
